# revision 20
# baseline (speedup 1.0000x reference)
"""Trainium2 Bass kernel for the DetNet-style nn module.

Strategy (pure data parallel, 8 cores x 256 samples):
  - All activations live in [feature(partition), sample(free)] layout.
  - Per-sample Psi matvecs run on the PE with Psi as (self-loading) stationary:
      forward  : lhsT = Psi^T chunks [s<=128, m=128], rhs = x column  -> Pe_x column
      transpose: lhsT = Psi   chunks [m=128, s<=128], rhs = u column  -> q-corr column
    Both Psi layouts are resident in SBUF (bf16), per processing wave.
  - MLP (W1/W2/W3) are shared-weight matmuls with batch on the moving free dim.
  - Softmax / constellation mapping on DVE/ACT with stride-tricks APs.
  - Hardware For_i loop over layers (2-layer unrolled body, ping-pong weights).
Outputs are written in device layout and transposed on the host.
"""

import sys
from contextlib import ExitStack

import numpy as np
import ml_dtypes

sys.path.insert(0, "/opt/trn_rl_repo")

import concourse.bass as bass
import concourse.tile as tile
from concourse import bacc
from concourse import mybir
from concourse.bass import ds

FP = mybir.dt.float32
BF = mybir.dt.bfloat16
AF = mybir.ActivationFunctionType
ALU = mybir.AluOpType

L = 30
B = 64          # block_len
S = 72          # sym_len
OH = 4
VL = 128
ZL = 512
BATCH = 2048
NCORES = 8
PER = BATCH // NCORES      # 256
M2 = 2 * B                 # 128  (m dim)
S2 = 2 * S                 # 144  (s dim)
SLO = 128                  # forward contraction low chunk
SHI = S2 - SLO             # 16
Z2 = 2 * ZL                # 1024
NJ = Z2 // 128             # 8 z chunks
V2 = 2 * VL                # 256

bf = ml_dtypes.bfloat16

# mega weight-pack column map (bf16 columns)
C_W1A = 0
C_W1C = C_W1A + Z2
C_W1D = C_W1C + Z2
C_W1B = C_W1D + Z2
C_W2 = C_W1B + Z2            # + j*OH*S + k*S
C_W3 = C_W2 + NJ * OH * S    # + j*V2 + m2*128
C_B2 = C_W3 + NJ * V2        # row 0
C_B3 = C_B2 + OH * S         # row 0
C_F32 = C_B3 + V2            # fp32 region (bitcast), cols in fp32 units:
NF32 = 10                    # b1 (8) + dsc (2)
WCOLS = C_F32 + 2 * NF32


def build_program(nlayers, waves):
    """Build the Bass program. waves = list of per-wave sample counts."""
    nc = bacc.Bacc("TRN2", target_bir_lowering=False, debug=False)
    NW = len(waves)
    WMAX = max(waves)

    dram = {}

    def din(name, shape, dt):
        dram[name] = nc.dram_tensor(name, list(shape), dt, kind="ExternalInput")
        return dram[name]

    def dout(name, shape, dt):
        dram[name] = nc.dram_tensor(name, list(shape), dt, kind="ExternalOutput")
        return dram[name]

    pTe = [din(f"pTe{w}", (128, waves[w] * M2), BF) for w in range(NW)]
    pTo = [din(f"pTo{w}", (128, waves[w] * M2), BF) for w in range(NW)]
    pHi = [din(f"pHi{w}", (128, waves[w] * M2), BF) for w in range(NW)]
    nEe = [din(f"nEe{w}", (128, waves[w] * S2), BF) for w in range(NW)]
    nEo = [din(f"nEo{w}", (128, waves[w] * S2), BF) for w in range(NW)]
    yeod = [din(f"yeo{w}", (nlayers, B, 2 * waves[w]), FP) for w in range(NW)]
    wpackd = din("wpack", (nlayers, 128, WCOLS), BF)
    mapd = din("mapp", (S, 2 * OH), FP)
    seld = din("sel", (64, 16), BF)

    xs_out = dout("xs_dev", (nlayers, NW, 176, WMAX), BF)
    xoh_out = dout("xoh_dev", (nlayers, NW, S, OH * WMAX), FP)

    with tile.TileContext(nc) as tc, ExitStack() as ctx:
        ppsi = ctx.enter_context(tc.tile_pool(name="psi", bufs=1))
        pact = ctx.enter_context(tc.tile_pool(name="act", bufs=1))
        pwt = ctx.enter_context(tc.tile_pool(name="wt", bufs=2))
        pscr = ctx.enter_context(tc.tile_pool(name="scr", bufs=1))
        ppsum = ctx.enter_context(tc.tile_pool(name="ps", bufs=1, space="PSUM"))
        ppsumz = ctx.enter_context(tc.tile_pool(name="psz", bufs=2, space="PSUM"))

        for wv, W in enumerate(waves):
            tpTe = ppsi.tile([128, W * M2], BF, tag="pTe")
            tpTo = ppsi.tile([128, W * M2], BF, tag="pTo")
            tpHi = ppsi.tile([128, W * M2], BF, tag="pHi")
            tnEe = ppsi.tile([128, W * S2], BF, tag="nEe")
            tnEo = ppsi.tile([128, W * S2], BF, tag="nEo")
            nc.sync.dma_start(tpTe[:], pTe[wv][:, :])
            nc.sync.dma_start(tpTo[:], pTo[wv][:, :])
            nc.sync.dma_start(tpHi[:], pHi[wv][:, :])
            nc.sync.dma_start(tnEe[:], nEe[wv][:, :])
            nc.sync.dma_start(tnEo[:], nEo[wv][:, :])

            x_lo = pact.tile([128, W], BF, tag="x_lo")
            x_hi = pact.tile([128, W], BF, tag="x_hi")
            vA = pact.tile([128, W], BF, tag="vA")
            vB = pact.tile([128, W], BF, tag="vB")
            xoh = pact.tile([S, OH * W], FP, tag="xoh")
            z = pact.tile([128, NJ * W], BF, tag="z")
            u_e = pact.tile([128, W], BF, tag="u_e")
            u_o = pact.tile([128, W], BF, tag="u_o")
            q_lo = pact.tile([128, W], BF, tag="q_lo")
            q_hi = pact.tile([16, W], BF, tag="q_hi")
            ones = pact.tile([1, W], BF, tag="ones")
            mapt = pact.tile([S, 2 * OH], FP, tag="mapt")
            selt = pact.tile([64, 16], BF, tag="selt")
            nc.sync.dma_start(selt[:], seld[:, :])

            nc.vector.memset(x_lo[:], 0.0)
            nc.vector.memset(x_hi[:], 0.0)
            nc.vector.memset(vA[:], 0.0)
            nc.vector.memset(vB[:], 0.0)
            nc.vector.memset(xoh[:], 0.0)
            nc.vector.memset(ones[:], 1.0)
            nc.sync.dma_start(mapt[:], mapd[:, :])

            def layer_body(l, dyn, wts, is_zero=False):
                (w1a, w1b, w1c, w1d, w2t, w3t, b1t, b2t, b3t, yet, yot, dsct) = wts

                ps_e = ppsum.tile([128, W], FP, tag="fwd_e")
                ps_o = ppsum.tile([128, W], FP, tag="fwd_o")
                if not is_zero:
                    for b in range(W):
                        nc.tensor.matmul(
                            ps_e[:, b : b + 1], tpTe[:, b * M2 : (b + 1) * M2],
                            x_lo[:, b : b + 1], start=True, stop=False)
                        nc.tensor.matmul(
                            ps_e[:, b : b + 1], tpHi[0:64, b * M2 : (b + 1) * M2],
                            x_hi[0:64, b : b + 1], start=False, stop=True)
                        nc.tensor.matmul(
                            ps_o[:, b : b + 1], tpTo[:, b * M2 : (b + 1) * M2],
                            x_lo[:, b : b + 1], start=True, stop=False)
                        nc.tensor.matmul(
                            ps_o[:, b : b + 1], tpHi[64:128, b * M2 : (b + 1) * M2],
                            x_hi[64:128, b : b + 1], start=False, stop=True)

                    sq_eA = pscr.tile([B, W], FP, tag="sq_eA")
                    sq_eB = pscr.tile([B, W], FP, tag="sq_eB")
                    sq_oA = pscr.tile([B, W], FP, tag="sq_oA")
                    sq_oB = pscr.tile([B, W], FP, tag="sq_oB")
                    nc.scalar.square(sq_eA[:], ps_e[0:B, :])
                    nc.scalar.square(sq_eB[:], ps_e[B:128, :])
                    nc.scalar.square(sq_oA[:], ps_o[0:B, :])
                    nc.scalar.square(sq_oB[:], ps_o[B:128, :])
                    we_t = pscr.tile([B, W], FP, tag="we")
                    wo_t = pscr.tile([B, W], FP, tag="wo")
                    nc.vector.tensor_tensor(we_t[:], sq_eA[:], sq_eB[:], op=ALU.add)
                    nc.vector.tensor_tensor(wo_t[:], sq_oA[:], sq_oB[:], op=ALU.add)
                    # w = (sql * d) - yscaled
                    nc.vector.scalar_tensor_tensor(
                        we_t[:], we_t[:], dsct[0:B, 0:1], yet[:],
                        op0=ALU.mult, op1=ALU.subtract)
                    nc.vector.scalar_tensor_tensor(
                        wo_t[:], wo_t[:], dsct[0:B, 1:2], yot[:],
                        op0=ALU.mult, op1=ALU.subtract)
                    nc.vector.tensor_tensor(u_e[0:B, :], ps_e[0:B, :], we_t[:], op=ALU.mult)
                    nc.vector.tensor_tensor(u_e[B:128, :], ps_e[B:128, :], we_t[:], op=ALU.mult)
                    nc.vector.tensor_tensor(u_o[0:B, :], ps_o[0:B, :], wo_t[:], op=ALU.mult)
                    nc.vector.tensor_tensor(u_o[B:128, :], ps_o[B:128, :], wo_t[:], op=ALU.mult)

                    ps_qlo = ppsum.tile([128, W], FP, tag="qlo")
                    ps_qhi = ppsum.tile([16, W], FP, tag="qhi")
                    nc.tensor.matmul(ps_qhi[:, :], selt[:, :], x_hi[0:64, :],
                                     start=True, stop=False)
                    for b in range(W):
                        nc.tensor.matmul(
                            ps_qlo[:, b : b + 1], tnEe[:, b * S2 : b * S2 + SLO],
                            u_e[:, b : b + 1], start=True, stop=False)
                        nc.tensor.matmul(
                            ps_qlo[:, b : b + 1], tnEo[:, b * S2 : b * S2 + SLO],
                            u_o[:, b : b + 1], start=False, stop=True)
                        nc.tensor.matmul(
                            ps_qhi[:, b : b + 1], tnEe[:, b * S2 + SLO : (b + 1) * S2],
                            u_e[:, b : b + 1], start=False, stop=False)
                        nc.tensor.matmul(
                            ps_qhi[:, b : b + 1], tnEo[:, b * S2 + SLO : (b + 1) * S2],
                            u_o[:, b : b + 1], start=False, stop=(b == W - 1))
                    nc.vector.tensor_tensor(q_lo[:], x_lo[:], ps_qlo[:], op=ALU.add)
                    nc.scalar.copy(q_hi[:], ps_qhi[:, :])

                for j in range(NJ):
                    ps_z = ppsumz.tile([128, W], FP, tag="z")
                    if not is_zero:
                        nc.tensor.matmul(ps_z[:, :], w1a[:, j * 128 : (j + 1) * 128],
                                         q_lo[:], start=True, stop=False)
                        nc.tensor.matmul(ps_z[:, :], w1b[:, j * 128 : (j + 1) * 128],
                                         q_hi[:], start=False, stop=False)
                        nc.tensor.matmul(ps_z[:, :], w1c[:, j * 128 : (j + 1) * 128],
                                         vA[:], start=False, stop=False)
                        nc.tensor.matmul(ps_z[:, :], w1d[:, j * 128 : (j + 1) * 128],
                                         vB[:], start=False, stop=True)
                        nc.scalar.activation(z[:, j * W : (j + 1) * W], ps_z[:, :],
                                             AF.Relu, bias=b1t[:, j : j + 1])
                    else:
                        nc.scalar.activation(
                            z[:, j * W : (j + 1) * W],
                            b1t[:, j : j + 1].broadcast_to([128, W]),
                            AF.Relu)

                ps_xoh = ppsum.tile([S, OH * W], FP, tag="xoh")
                for k in range(OH):
                    for j in range(NJ):
                        nc.tensor.matmul(
                            ps_xoh[:, k * W : (k + 1) * W],
                            w2t[j][:, k * S : (k + 1) * S],
                            z[:, j * W : (j + 1) * W],
                            start=(j == 0), stop=False)
                    nc.tensor.matmul(ps_xoh[:, k * W : (k + 1) * W],
                                     b2t[:, k * S : (k + 1) * S], ones[:],
                                     start=False, stop=True)
                nc.vector.tensor_tensor(xoh[:], xoh[:], ps_xoh[:, :], op=ALU.add)

                ps_v = ppsum.tile([128, 2 * W], FP, tag="v")
                for m2 in range(2):
                    for j in range(NJ):
                        nc.tensor.matmul(
                            ps_v[:, m2 * W : (m2 + 1) * W],
                            w3t[j][:, m2 * 128 : (m2 + 1) * 128],
                            z[:, j * W : (j + 1) * W],
                            start=(j == 0), stop=False)
                    nc.tensor.matmul(ps_v[:, m2 * W : (m2 + 1) * W],
                                     b3t[:, m2 * 128 : (m2 + 1) * 128], ones[:],
                                     start=False, stop=True)
                nc.vector.tensor_tensor(vA[:], vA[:], ps_v[:, 0:W], op=ALU.add)
                nc.vector.tensor_tensor(vB[:], vB[:], ps_v[:, W : 2 * W], op=ALU.add)

                # oh_2_sym
                kin = xoh[:].rearrange("s (k b) -> s b k", k=OH)
                mx = pscr.tile([S, W], FP, tag="mx")
                nc.vector.tensor_reduce(mx[:], kin, axis=mybir.AxisListType.X, op=ALU.max)
                ex = pscr.tile([S, OH * W], FP, tag="ex")
                xoh3 = xoh[:].rearrange("s (k b) -> s k b", k=OH)
                ex3 = ex[:].rearrange("s (k b) -> s k b", k=OH)
                mxb = mx[:].unsqueeze(1).broadcast_to([S, OH, W])
                nc.vector.tensor_tensor(ex3, xoh3, mxb, op=ALU.subtract)
                nc.scalar.activation(ex[:], ex[:], AF.Exp)
                sm = pscr.tile([S, W], FP, tag="sm")
                nc.vector.tensor_reduce(sm[:], ex[:].rearrange("s (k b) -> s b k", k=OH),
                                        axis=mybir.AxisListType.X, op=ALU.add)
                rc = pscr.tile([S, W], FP, tag="rc")
                nc.vector.reciprocal(rc[:], sm[:])
                tw = pscr.tile([S, OH * W], FP, tag="tw")
                wre = pscr.tile([S, W], FP, tag="wre")
                wim = pscr.tile([S, W], FP, tag="wim")
                tw3 = tw[:].rearrange("s (k b) -> s k b", k=OH)
                mre_b = mapt[:, 0:OH].unsqueeze(2).broadcast_to([S, OH, W])
                mim_b = mapt[:, OH : 2 * OH].unsqueeze(2).broadcast_to([S, OH, W])
                nc.vector.tensor_tensor(tw3, ex3, mre_b, op=ALU.mult)
                nc.vector.tensor_reduce(wre[:], tw[:].rearrange("s (k b) -> s b k", k=OH),
                                        axis=mybir.AxisListType.X, op=ALU.add)
                nc.vector.tensor_tensor(tw3, ex3, mim_b, op=ALU.mult)
                nc.vector.tensor_reduce(wim[:], tw[:].rearrange("s (k b) -> s b k", k=OH),
                                        axis=mybir.AxisListType.X, op=ALU.add)
                nc.vector.tensor_tensor(x_lo[0:64, :], wre[0:64, :], rc[0:64, :], op=ALU.mult)
                nc.vector.tensor_tensor(x_lo[64:128, :], wim[0:64, :],
                                        rc[0:64, :], op=ALU.mult)
                nc.vector.tensor_tensor(x_hi[0:8, :], wre[64:S, :],
                                        rc[64:S, :], op=ALU.mult)
                nc.vector.tensor_tensor(x_hi[32:40, :], wim[64:S, :],
                                        rc[64:S, :], op=ALU.mult)
                nc.vector.tensor_copy(x_hi[64:128, :], x_hi[0:64, :])

                # outputs (device layout)
                if dyn:
                    xsl = xs_out[ds(l, 1)]
                    xol = xoh_out[ds(l, 1)]
                    nc.gpsimd.dma_start(xsl[:, wv : wv + 1, 0:SLO, 0:W], x_lo[:])
                    nc.gpsimd.dma_start(xsl[:, wv : wv + 1, SLO:176, 0:W], x_hi[0:48, :])
                    nc.scalar.dma_start(xol[:, wv : wv + 1, :, 0 : OH * W], xoh[:])
                else:
                    nc.gpsimd.dma_start(xs_out[l, wv, 0:SLO, 0:W], x_lo[:])
                    nc.gpsimd.dma_start(xs_out[l, wv, SLO:176, 0:W], x_hi[0:48, :])
                    nc.scalar.dma_start(xoh_out[l, wv, :, 0 : OH * W], xoh[:])

            def load_weights(l, dyn, first=False):
                mg = pwt.tile([128, WCOLS], BF, tag="wmega")
                yeot = pwt.tile([B, 2 * W], FP, tag="yeo")
                yet = yeot[:, 0:W]
                yot = yeot[:, W : 2 * W]
                if dyn:
                    nc.sync.dma_start(mg[:], wpackd[ds(l, 1)])
                    nc.sync.dma_start(yeot[:], yeod[wv][ds(l, 1)])
                else:
                    nc.sync.dma_start(mg[:], wpackd[l])
                    if not first:
                        nc.sync.dma_start(yeot[:], yeod[wv][l])
                f32 = mg[:, C_F32 : C_F32 + 2 * NF32].bitcast(FP)
                w1a = mg[:, C_W1A : C_W1A + Z2]
                w1b = mg[0:16, C_W1B : C_W1B + Z2]
                w1c = mg[:, C_W1C : C_W1C + Z2]
                w1d = mg[:, C_W1D : C_W1D + Z2]
                w2t = [mg[:, C_W2 + j * OH * S : C_W2 + (j + 1) * OH * S]
                       for j in range(NJ)]
                w3t = [mg[:, C_W3 + j * V2 : C_W3 + (j + 1) * V2] for j in range(NJ)]
                b1t = f32[:, 0:8]
                dsct = f32[:, 8:10]
                b2t = mg[0:1, C_B2 : C_B2 + OH * S]
                b3t = mg[0:1, C_B3 : C_B3 + V2]
                return (w1a, w1b, w1c, w1d, w2t, w3t, b1t, b2t, b3t, yet, yot, dsct)

            wts = load_weights(0, False, first=True)
            layer_body(0, False, wts, is_zero=True)

            if nlayers > 2:
                n_loop = (nlayers - 2) // 2 * 2
                with tc.For_i(1, 1 + n_loop, 2) as lv:
                    wts = load_weights(lv, True)
                    layer_body(lv, True, wts)
                    wts2 = load_weights(lv + 1, True)
                    layer_body(lv + 1, True, wts2)
                for lt in range(1 + n_loop, nlayers):
                    wts = load_weights(lt, False)
                    layer_body(lt, False, wts)
            else:
                for lt in range(1, nlayers):
                    wts = load_weights(lt, False)
                    layer_body(lt, False, wts)

    nc.compile()
    return nc, dram


# ====================== host-side packing ======================

def pack_core_inputs(sl, nlayers, waves, y_e, y_o, Psi_e, Psi_o,
                     mapp_re, mapp_im, W1, b1, W2, b2, W3, b3, d1, d2, d4):
    NW = len(waves)
    woff = np.cumsum([0] + list(waves))
    im = {}
    # device s ordering: lo = [re 0:64 | im 0:64], tails re[64:72], im[64:72]
    lo_idx = np.concatenate([np.arange(0, 64), np.arange(S, S + 64)])
    tre_idx = np.arange(64, S)
    tim_idx = np.arange(S + 64, S2)
    nat_idx = np.concatenate([lo_idx, tre_idx, tim_idx])
    for w in range(NW):
        Wn = waves[w]
        bs = sl[woff[w] : woff[w] + Wn]
        Pe = Psi_e[bs]
        Po = Psi_o[bs]
        im[f"pTe{w}"] = np.ascontiguousarray(
            Pe[:, :, lo_idx].transpose(2, 0, 1).reshape(SLO, Wn * M2)).astype(bf)
        im[f"pTo{w}"] = np.ascontiguousarray(
            Po[:, :, lo_idx].transpose(2, 0, 1).reshape(SLO, Wn * M2)).astype(bf)
        hi = np.zeros((128, Wn * M2), np.float32)
        hi[0:8] = Pe[:, :, tre_idx].transpose(2, 0, 1).reshape(8, Wn * M2)
        hi[32:40] = Pe[:, :, tim_idx].transpose(2, 0, 1).reshape(8, Wn * M2)
        hi[64:72] = Po[:, :, tre_idx].transpose(2, 0, 1).reshape(8, Wn * M2)
        hi[96:104] = Po[:, :, tim_idx].transpose(2, 0, 1).reshape(8, Wn * M2)
        im[f"pHi{w}"] = hi.astype(bf)
        im[f"nEe{w}"] = np.ascontiguousarray(
            Pe[:, :, nat_idx].transpose(1, 0, 2).reshape(M2, Wn * S2)).astype(bf)
        im[f"nEo{w}"] = np.ascontiguousarray(
            Po[:, :, nat_idx].transpose(1, 0, 2).reshape(M2, Wn * S2)).astype(bf)
        yeo = np.concatenate([
            d1[:nlayers, None, None] * y_e[bs].T[None, :, :],
            d2[:nlayers, None, None] * y_o[bs].T[None, :, :]], axis=2)
        im[f"yeo{w}"] = np.ascontiguousarray(yeo).astype(np.float32)
    # ---- mega weight pack ----
    wp = np.zeros((nlayers, 128, WCOLS), np.float32)
    W1T = W1[:nlayers].transpose(0, 2, 1)           # [l, 400, 1024]
    wp[:, :, C_W1A : C_W1A + Z2] = W1T[:, lo_idx]
    wp[:, 0:8, C_W1B : C_W1B + Z2] = W1T[:, tre_idx]
    wp[:, 8:16, C_W1B : C_W1B + Z2] = W1T[:, tim_idx]
    wp[:, :, C_W1C : C_W1C + Z2] = W1T[:, S2 : S2 + 128]
    wp[:, :, C_W1D : C_W1D + Z2] = W1T[:, S2 + 128 : S2 + 256]
    W2r = W2[:nlayers].reshape(nlayers, S, OH, Z2)
    wp[:, :, C_W2 : C_W2 + NJ * OH * S] = (
        W2r.transpose(0, 3, 2, 1).reshape(nlayers, NJ, 128, OH * S)
        .transpose(0, 2, 1, 3).reshape(nlayers, 128, NJ * OH * S))
    wp[:, :, C_W3 : C_W3 + NJ * V2] = (
        W3[:nlayers].transpose(0, 2, 1).reshape(nlayers, NJ, 128, V2)
        .transpose(0, 2, 1, 3).reshape(nlayers, 128, NJ * V2))
    wp[:, 0, C_B2 : C_B2 + OH * S] = b2[:nlayers].reshape(
        nlayers, S, OH).transpose(0, 2, 1).reshape(nlayers, OH * S)
    wp[:, 0, C_B3 : C_B3 + V2] = b3[:nlayers]
    wpack = wp.astype(bf)
    f32blk = np.zeros((nlayers, 128, NF32), np.float32)
    f32blk[:, :, 0:8] = b1[:nlayers].reshape(nlayers, NJ, 128).transpose(0, 2, 1)
    f32blk[:, :, 8] = np.broadcast_to(d2[:nlayers, None], (nlayers, 128))
    f32blk[:, :, 9] = np.broadcast_to(d4[:nlayers, None], (nlayers, 128))
    wpack[:, :, C_F32 : C_F32 + 2 * NF32] = (
        np.ascontiguousarray(f32blk).view(bf).reshape(nlayers, 128, 2 * NF32))
    im["wpack"] = np.ascontiguousarray(wpack)
    sel = np.zeros((64, 16), np.float32)
    for r in range(8):
        sel[r, r] = 1.0
        sel[32 + r, 8 + r] = 1.0
    im["sel"] = sel.astype(bf)
    im["mapp"] = np.ascontiguousarray(np.concatenate(
        [np.broadcast_to(mapp_re[None, :], (S, OH)),
         np.broadcast_to(mapp_im[None, :], (S, OH))], axis=1)).astype(np.float32)
    return im


def unpack_core_outputs(res, nlayers, waves):
    NW = len(waves)
    WMAX = max(waves)
    xs_d = np.asarray(res["xs_dev"]).astype(np.float32)
    xoh_d = np.asarray(res["xoh_dev"]).astype(np.float32)
    lo_idx = np.concatenate([np.arange(0, 64), np.arange(S, S + 64)])
    xs = []
    xoh = []
    for w in range(NW):
        Wn = waves[w]
        xd = xs_d[:, w, :, 0:Wn].transpose(0, 2, 1)     # [l, Wn, 176]
        nl = xd.shape[0]
        xw = np.zeros((nl, Wn, S2), np.float32)
        xw[..., lo_idx] = xd[..., 0:SLO]
        xw[..., 64:S] = xd[..., 128:136]
        xw[..., S + 64 : S2] = xd[..., 160:168]
        xs.append(xw)
        xo = xoh_d[:, w][:, :, 0 : OH * Wn].reshape(nlayers, S, OH, Wn)
        xoh.append(xo.transpose(0, 3, 1, 2).reshape(nlayers, Wn, S * OH))
    return np.concatenate(xs, axis=1), np.concatenate(xoh, axis=1)


_prog_cache = {}
last_run = None


def _get_program(nlayers, waves):
    key = (nlayers, tuple(waves))
    if key not in _prog_cache:
        _prog_cache[key] = build_program(nlayers, waves)
    return _prog_cache[key]


def kernel(y_e, y_o, Psi_e, Psi_o, mapp_re, mapp_im,
           W1, b1, W2, b2, W3, b3, d1, d2, d4):
    from concourse.bass_utils import run_bass_kernel_spmd

    args = [np.asarray(a, np.float32) for a in
            (y_e, y_o, Psi_e, Psi_o, mapp_re, mapp_im,
             W1, b1, W2, b2, W3, b3, d1, d2, d4)]
    waves = [112, 112, 32]
    nc, _ = _get_program(L, waves)
    in_maps = []
    for c in range(NCORES):
        sl = np.arange(c * PER, (c + 1) * PER)
        in_maps.append(pack_core_inputs(sl, L, waves, *args))
    import os
    trace = bool(os.environ.get("DETNET_TRACE"))
    out = run_bass_kernel_spmd(nc, in_maps, core_ids=list(range(NCORES)),
                               trace=trace)
    global last_run
    last_run = out
    xs_parts, xoh_parts = [], []
    for c in range(NCORES):
        xs_c, xoh_c = unpack_core_outputs(out.results[c], L, waves)
        xs_parts.append(xs_c)
        xoh_parts.append(xoh_c)
    xs = np.concatenate(xs_parts, axis=1).astype(np.float32)
    xohs = np.concatenate(xoh_parts, axis=1).astype(np.float32)
    return xs, xohs
